# revision 10
# baseline (speedup 1.0000x reference)
"""One-hot-matmul embedding lookup for TRN2: out[i] = feature_array[int(x[i,0])].

Data-parallel over N across 8 NeuronCores; the [512, 64] table is replicated.

Per core (25000 rows padded to 25088 = 196 tiles of 128 rows, in 25 groups
of 8 tiles = one PSUM bank each):
  - x is pre-replicated host-side to [128, 25088] fp16 (ids < 2048 are
    exact in fp16), so the SBUF load is a plain contiguous HWDGE DMA.
    A partition-stride-0 broadcast DMA lowers to SWDGE whose gpsimd
    descriptor generation (~1.2us/group) was the previous bottleneck.
  - DVE builds the transposed one-hot oh[c, j*128+m] = (x[128(8g+j)+m] ==
    c + 128h) for the 4 case-chunks h via tensor_tensor(is_equal) against
    a replicated fp16 iota table (all operands fp16+SBUF for the fast
    DVE mode).
  - PE accumulates psum[m, d] += oh_h^T @ feat_chunk_h (fp16, 64-col
    moving operand; 4 matmuls per 128-row tile).
  - The scalar engine drains each PSUM bank (8 tiles) to SBUF; x-chunk
    loads and writeback DMAs alternate between the gpsimd and sync queues.

Raw bass (not TileContext): this walrus build rejects instructions with
more than one semaphore wait, so cross-engine dependencies are split into
single-wait `wait_ge` sequencer NOPs on the consuming engine.

fp16 table rounding gives rel err ~2^-11, far under the 2e-2 gate.
"""

import numpy as np

N = 200_000
C = 512
D = 64
NCORES = 8
NS = N // NCORES  # 25000
P = 128
T = 196  # tiles of 128 rows per core
NSP = P * T  # 25088 padded rows per core
G = 8  # tiles per PSUM bank / drain group
GP = G * P  # columns per group
NG = (T + G - 1) // G  # 25 groups; last has 4 tiles
B_OH = 3  # one-hot buffers
B_PS = 4  # psum banks
B_OSB = 3  # output staging buffers

_RUN_OPTS: dict = {}
_LAST_RESULT = None
_LAST_IN_MAPS = None
_NC_CACHE = None


def _build():
    global _NC_CACHE
    if _NC_CACHE is not None:
        return _NC_CACHE
    import concourse.bass as bass
    import concourse.mybir as mybir
    from contextlib import ExitStack

    f16 = mybir.dt.float16
    f32 = mybir.dt.float32
    EQ = mybir.AluOpType.is_equal

    nc = bass.Bass()
    xr = nc.dram_tensor("xr", [P, NSP], f16, kind="ExternalInput")
    feat = nc.dram_tensor("feat", [P, 4 * D], f16, kind="ExternalInput")
    iot = nc.dram_tensor("iot", [P, 4], f32, kind="ExternalInput")
    out = nc.dram_tensor("out", [NSP, D], f32, kind="ExternalOutput")

    def sg_of(g):
        return min(G, T - g * G)

    with ExitStack() as ctx:
        sb = ctx.enter_context
        feat_sb = sb(nc.sbuf_tensor("feat_sb", [P, 4 * D], f16))
        iot_sb = sb(nc.sbuf_tensor("iot_sb", [P, 4], f32))
        xrep = sb(nc.sbuf_tensor("xrep", [P, NSP], f16))
        oh = [sb(nc.sbuf_tensor(f"oh{b}", [P, 4, G, P], f16)) for b in range(B_OH)]
        osb = [sb(nc.sbuf_tensor(f"osb{b}", [P, G * D], f32)) for b in range(B_OSB)]
        ps = [sb(nc.psum_tensor(f"ps{b}", [P, G * D], f32)) for b in range(B_PS)]

        s_in_g = sb(nc.semaphore("s_in_g"))
        s_in_s = sb(nc.semaphore("s_in_s"))
        s_cmp = sb(nc.semaphore("s_cmp"))
        s_mm = sb(nc.semaphore("s_mm"))
        s_drain = sb(nc.semaphore("s_drain"))
        s_wb_g = sb(nc.semaphore("s_wb_g"))
        s_wb_s = sb(nc.semaphore("s_wb_s"))

        block = ctx.enter_context(nc.Block())

        def out_view(g):
            x0, sg = g * GP, sg_of(g)
            return out[x0 : x0 + sg * P, :].rearrange("(p j) d -> p (j d)", p=P)

        @block.sync
        def _(sync):
            sync.dma_start(out=feat_sb[:], in_=feat[:]).then_inc(s_in_s, 16)
            sync.dma_start(out=iot_sb[:], in_=iot[:]).then_inc(s_in_s, 16)
            for g in range(1, NG, 2):
                c0, cn = g * GP, sg_of(g) * P
                sync.dma_start(
                    out=xrep[:, c0 : c0 + cn], in_=xr[:, c0 : c0 + cn]
                ).then_inc(s_in_s, 16)
            for g in range(1, NG, 2):
                sync.wait_ge(s_drain, g + 1)
                sync.dma_start(
                    out=out_view(g), in_=osb[g % B_OSB][:, : sg_of(g) * D]
                ).then_inc(s_wb_s, 16)

        @block.gpsimd
        def _(gpsimd):
            for g in range(0, NG, 2):
                c0, cn = g * GP, sg_of(g) * P
                gpsimd.dma_start(
                    out=xrep[:, c0 : c0 + cn], in_=xr[:, c0 : c0 + cn]
                ).then_inc(s_in_g, 16)
            for g in range(0, NG, 2):
                gpsimd.wait_ge(s_drain, g + 1)
                gpsimd.dma_start(
                    out=out_view(g), in_=osb[g % B_OSB][:, : sg_of(g) * D]
                ).then_inc(s_wb_g, 16)

        @block.vector
        def _(vector):
            vector.wait_ge(s_in_s, 32)  # feat + iota loaded
            for g in range(NG):
                sg = sg_of(g)
                if g % 2 == 0:
                    vector.wait_ge(s_in_g, 16 * (g // 2 + 1))
                else:
                    vector.wait_ge(s_in_s, 16 * (2 + (g + 1) // 2))
                if g >= B_OH:
                    vector.wait_ge(s_mm, g - B_OH + 1)
                x0 = g * GP
                for h in range(4):
                    i = vector.tensor_scalar(
                        oh[g % B_OH][:, h, :sg, :],
                        xrep[:, x0 : x0 + sg * P],
                        iot_sb[:, h : h + 1],
                        None,
                        EQ,
                    )
                    if h == 3:
                        i.then_inc(s_cmp, 1)

        @block.tensor
        def _(tensor):
            tensor.wait_ge(s_in_s, 16)  # feat loaded
            for g in range(NG):
                sg = sg_of(g)
                if g >= B_PS:
                    tensor.wait_ge(s_drain, g - B_PS + 1)
                tensor.wait_ge(s_cmp, g + 1)
                for j in range(sg):
                    for h in range(4):
                        i = tensor.matmul(
                            ps[g % B_PS][:, j * D : (j + 1) * D],
                            oh[g % B_OH][:, h, j, :],
                            feat_sb[:, h * D : (h + 1) * D],
                            start=(h == 0),
                            stop=(h == 3),
                        )
                        if j == sg - 1 and h == 3:
                            i.then_inc(s_mm, 1)

        @block.scalar
        def _(scalar):
            for g in range(NG):
                sg = sg_of(g)
                if g >= B_OSB:
                    q = g - B_OSB
                    scalar.wait_ge(
                        s_wb_g if q % 2 == 0 else s_wb_s, 16 * (q // 2 + 1)
                    )
                scalar.wait_ge(s_mm, g + 1)
                scalar.copy(
                    osb[g % B_OSB][:, : sg * D], ps[g % B_PS][:, : sg * D]
                ).then_inc(s_drain, 1)

    _NC_CACHE = nc
    return nc


def kernel(x, feature_array):
    global _LAST_RESULT, _LAST_IN_MAPS
    from concourse.bass_utils import run_bass_kernel_spmd

    nc = _build()
    xs = np.asarray(x).reshape(-1).astype(np.float16)  # ids < 512: exact in fp16
    feat = np.asarray(feature_array, dtype=np.float32)
    # feat16[c2, h*64+d] = feat[128h + c2, d]
    feat16 = (
        feat.reshape(4, P, D).transpose(1, 0, 2).reshape(P, 4 * D).astype(np.float16)
    )
    iot = (
        np.arange(P, dtype=np.float32)[:, None]
        + np.arange(4, dtype=np.float32)[None, :] * P
    ).astype(np.float32)

    in_maps = []
    for i in range(NCORES):
        xp = np.zeros(NSP, dtype=np.float16)
        xp[:NS] = xs[i * NS : (i + 1) * NS]
        # within each group, transpose (p, j) -> (j, p) so the device's
        # writeback lands rows back in original order contiguously
        nfull = (NG - 1) * GP
        head = xp[:nfull].reshape(NG - 1, P, G).transpose(0, 2, 1).reshape(-1)
        tail = xp[nfull:].reshape(P, T - (NG - 1) * G).T.reshape(-1)
        xcol = np.concatenate([head, tail])[None, :]
        xrep = np.ascontiguousarray(np.broadcast_to(xcol, (P, NSP)))
        in_maps.append({"xr": xrep, "feat": feat16, "iot": iot})
    _LAST_IN_MAPS = in_maps
    res = run_bass_kernel_spmd(nc, in_maps, core_ids=list(range(NCORES)), **_RUN_OPTS)
    _LAST_RESULT = res
    return np.concatenate([r["out"][:NS] for r in res.results], axis=0)


# revision 11
# speedup vs baseline: 1.1259x; 1.1259x over previous
"""One-hot-matmul embedding lookup for TRN2: out[i] = feature_array[int(x[i,0])].

Data-parallel over N across 8 NeuronCores; the [512, 64] table is replicated.

Per core (25000 rows padded to 25088 = 196 tiles of 128 rows, in 25 groups
of 8 tiles = one PSUM bank each):
  - x is pre-replicated host-side to [128, 25088] fp16 (ids < 2048 are
    exact in fp16), so the SBUF load is a plain contiguous HWDGE DMA.
    A partition-stride-0 broadcast DMA lowers to SWDGE whose gpsimd
    descriptor generation (~1.2us/group) was the previous bottleneck.
  - DVE builds the transposed one-hot oh[c, j*128+m] = (x[128(8g+j)+m] ==
    c + 128h) for the 4 case-chunks h via tensor_tensor(is_equal) against
    a replicated fp16 iota table (all operands fp16+SBUF for the fast
    DVE mode).
  - PE accumulates psum[m, d] += oh_h^T @ feat_chunk_h (fp16, 64-col
    moving operand; 4 matmuls per 128-row tile).
  - The scalar engine drains each PSUM bank (8 tiles) to SBUF; x-chunk
    loads and writeback DMAs alternate between the gpsimd and sync queues.

Raw bass (not TileContext): this walrus build rejects instructions with
more than one semaphore wait, so cross-engine dependencies are split into
single-wait `wait_ge` sequencer NOPs on the consuming engine.

fp16 table rounding gives rel err ~2^-11, far under the 2e-2 gate.
"""

import numpy as np

N = 200_000
C = 512
D = 64
NCORES = 8
NS = N // NCORES  # 25000
P = 128
T = 196  # tiles of 128 rows per core
NSP = P * T  # 25088 padded rows per core
G = 8  # tiles per PSUM bank / drain group
GP = G * P  # columns per group
NG = (T + G - 1) // G  # 25 groups; last has 4 tiles
B_OH = 6  # one-hot buffers
B_PS = 8  # psum banks
B_OSB = 6  # output staging buffers

_RUN_OPTS: dict = {}
_LAST_RESULT = None
_LAST_IN_MAPS = None
_NC_CACHE = None


def _build():
    global _NC_CACHE
    if _NC_CACHE is not None:
        return _NC_CACHE
    import concourse.bass as bass
    import concourse.mybir as mybir
    from contextlib import ExitStack

    f16 = mybir.dt.float16
    f32 = mybir.dt.float32
    EQ = mybir.AluOpType.is_equal

    nc = bass.Bass()
    xr = nc.dram_tensor("xr", [P, NSP], f16, kind="ExternalInput")
    feat = nc.dram_tensor("feat", [P, 4 * D], f16, kind="ExternalInput")
    iot = nc.dram_tensor("iot", [P, 4], f32, kind="ExternalInput")
    out = nc.dram_tensor("out", [NSP, D], f32, kind="ExternalOutput")

    def sg_of(g):
        return min(G, T - g * G)

    with ExitStack() as ctx:
        sb = ctx.enter_context
        feat_sb = sb(nc.sbuf_tensor("feat_sb", [P, 4 * D], f16))
        iot_sb = sb(nc.sbuf_tensor("iot_sb", [P, 4], f32))
        xrep = sb(nc.sbuf_tensor("xrep", [P, NSP], f16))
        oh = [sb(nc.sbuf_tensor(f"oh{b}", [P, 4, G, P], f16)) for b in range(B_OH)]
        osb = [sb(nc.sbuf_tensor(f"osb{b}", [P, G * D], f32)) for b in range(B_OSB)]
        ps = [sb(nc.psum_tensor(f"ps{b}", [P, G * D], f32)) for b in range(B_PS)]

        s_in_g = sb(nc.semaphore("s_in_g"))
        s_in_s = sb(nc.semaphore("s_in_s"))
        s_cmp = sb(nc.semaphore("s_cmp"))
        s_mm = sb(nc.semaphore("s_mm"))
        s_drain = sb(nc.semaphore("s_drain"))
        s_wb_g = sb(nc.semaphore("s_wb_g"))
        s_wb_s = sb(nc.semaphore("s_wb_s"))

        block = ctx.enter_context(nc.Block())

        def out_view(g):
            x0, sg = g * GP, sg_of(g)
            return out[x0 : x0 + sg * P, :].rearrange("(p j) d -> p (j d)", p=P)

        @block.sync
        def _(sync):
            sync.dma_start(out=feat_sb[:], in_=feat[:]).then_inc(s_in_s, 16)
            sync.dma_start(out=iot_sb[:], in_=iot[:]).then_inc(s_in_s, 16)
            for g in range(1, NG, 2):
                c0, cn = g * GP, sg_of(g) * P
                sync.dma_start(
                    out=xrep[:, c0 : c0 + cn], in_=xr[:, c0 : c0 + cn]
                ).then_inc(s_in_s, 16)
            for g in range(1, NG, 2):
                sync.wait_ge(s_drain, g + 1)
                sync.dma_start(
                    out=out_view(g), in_=osb[g % B_OSB][:, : sg_of(g) * D]
                ).then_inc(s_wb_s, 16)

        @block.gpsimd
        def _(gpsimd):
            for g in range(0, NG, 2):
                c0, cn = g * GP, sg_of(g) * P
                gpsimd.dma_start(
                    out=xrep[:, c0 : c0 + cn], in_=xr[:, c0 : c0 + cn]
                ).then_inc(s_in_g, 16)
            for g in range(0, NG, 2):
                gpsimd.wait_ge(s_drain, g + 1)
                gpsimd.dma_start(
                    out=out_view(g), in_=osb[g % B_OSB][:, : sg_of(g) * D]
                ).then_inc(s_wb_g, 16)

        @block.vector
        def _(vector):
            vector.wait_ge(s_in_s, 32)  # feat + iota loaded
            for g in range(NG):
                sg = sg_of(g)
                if g % 2 == 0:
                    vector.wait_ge(s_in_g, 16 * (g // 2 + 1))
                else:
                    vector.wait_ge(s_in_s, 16 * (2 + (g + 1) // 2))
                if g >= B_OH:
                    vector.wait_ge(s_mm, g - B_OH + 1)
                x0 = g * GP
                for h in range(4):
                    i = vector.tensor_scalar(
                        oh[g % B_OH][:, h, :sg, :],
                        xrep[:, x0 : x0 + sg * P],
                        iot_sb[:, h : h + 1],
                        None,
                        EQ,
                    )
                    if h == 3:
                        i.then_inc(s_cmp, 1)

        @block.tensor
        def _(tensor):
            tensor.wait_ge(s_in_s, 16)  # feat loaded
            for g in range(NG):
                sg = sg_of(g)
                if g >= B_PS:
                    tensor.wait_ge(s_drain, g - B_PS + 1)
                tensor.wait_ge(s_cmp, g + 1)
                for j in range(sg):
                    for h in range(4):
                        i = tensor.matmul(
                            ps[g % B_PS][:, j * D : (j + 1) * D],
                            oh[g % B_OH][:, h, j, :],
                            feat_sb[:, h * D : (h + 1) * D],
                            start=(h == 0),
                            stop=(h == 3),
                        )
                        if j == sg - 1 and h == 3:
                            i.then_inc(s_mm, 1)

        @block.scalar
        def _(scalar):
            for g in range(NG):
                sg = sg_of(g)
                if g >= B_OSB:
                    q = g - B_OSB
                    scalar.wait_ge(
                        s_wb_g if q % 2 == 0 else s_wb_s, 16 * (q // 2 + 1)
                    )
                scalar.wait_ge(s_mm, g + 1)
                scalar.copy(
                    osb[g % B_OSB][:, : sg * D], ps[g % B_PS][:, : sg * D]
                ).then_inc(s_drain, 1)

    _NC_CACHE = nc
    return nc


def kernel(x, feature_array):
    global _LAST_RESULT, _LAST_IN_MAPS
    from concourse.bass_utils import run_bass_kernel_spmd

    nc = _build()
    xs = np.asarray(x).reshape(-1).astype(np.float16)  # ids < 512: exact in fp16
    feat = np.asarray(feature_array, dtype=np.float32)
    # feat16[c2, h*64+d] = feat[128h + c2, d]
    feat16 = (
        feat.reshape(4, P, D).transpose(1, 0, 2).reshape(P, 4 * D).astype(np.float16)
    )
    iot = (
        np.arange(P, dtype=np.float32)[:, None]
        + np.arange(4, dtype=np.float32)[None, :] * P
    ).astype(np.float32)

    in_maps = []
    for i in range(NCORES):
        xp = np.zeros(NSP, dtype=np.float16)
        xp[:NS] = xs[i * NS : (i + 1) * NS]
        # within each group, transpose (p, j) -> (j, p) so the device's
        # writeback lands rows back in original order contiguously
        nfull = (NG - 1) * GP
        head = xp[:nfull].reshape(NG - 1, P, G).transpose(0, 2, 1).reshape(-1)
        tail = xp[nfull:].reshape(P, T - (NG - 1) * G).T.reshape(-1)
        xcol = np.concatenate([head, tail])[None, :]
        xrep = np.ascontiguousarray(np.broadcast_to(xcol, (P, NSP)))
        in_maps.append({"xr": xrep, "feat": feat16, "iot": iot})
    _LAST_IN_MAPS = in_maps
    res = run_bass_kernel_spmd(nc, in_maps, core_ids=list(range(NCORES)), **_RUN_OPTS)
    _LAST_RESULT = res
    return np.concatenate([r["out"][:NS] for r in res.results], axis=0)
